# revision 29
# baseline (speedup 1.0000x reference)
"""Trainium2 Bass kernel for the audio-visual attention model.

Math (per (b,t) sample, BT = 32*64 = 2048 of them):
    V   = video[b,t]                              # [48, 512]
    v   = relu(V @ W_video.T + b_video)           # [48, 512]
    a   = relu(audio[b,t] @ W_audio.T + b_audio)  # [512]
    inter   = a @ W_g.T                           # [48]
    content = v @ W_v.T + inter[:, None]          # [48, 48]
    z   = tanh(content) @ W_h.T                   # [48]
    alpha = softmax(z)
    out = alpha @ V                               # [512]

Strategy: data-parallel over BT across 8 cores (256 samples each, R = 256*48
= 12288 video rows per core).  The host pre-transposes the video shard to
V.T [512, 12288], pre-arranges weights into device layouts, and runs the
matmul chain in fp16 (~7e-4 rel err, 1 row/cycle on the PE).

Per-core pipeline over 4 superblocks of 3072 rows, processed in 12
double-sub-blocks (DSUB=1024 cols, two 512-col halves):
    vT.relu   relu(W_video.T^T @ V.T + b_video)      PE + ACT
              -- per m-chunk, the two 512-col halves accumulate into one
                 2-bank PSUM tile; ONE fused relu covers 1024 cols
    content.T W_v.T^T @ vT.relu + ones^T @ inter     PE [48, cols]
              -- col half A -> array col group 0-47, half B -> 64-111
                 (tile_position col tiling): the two 5-matmul chains run
                 CONCURRENTLY, halving content PE time
    tanh      ONE fused op over partitions 0-111: half A lands in
              th[0:48, 512p:+512], half B in th[64:112, 512p:+512]
              (rows 48-63 hold junk, never read)
    z         two row-tiled matmuls (rows 0-47 / 64-111) -> one 2-bank
              PSUM tile, CONCURRENT on the PE                PE [128, cols]
    ez=exp(z) ONE fused op per DSUB                          ACT -> fp16
    weighted  V.T * ez in place                              DVE 2x mode
    cT groups halving-tree adds + reduce per 48-col group    DVE
              -- emitted in <=16-group chunks as soon as the covering
                 columns are multiplied; the last superblock's chunks are
                 staged (10/11/11 groups) so only ~2 small chunks trail
                 the final matmul, each gated on a half-DSUB mul
    denom     48-group sums of ez row 0                      DVE (per chunk)
Outputs (unnormalized cT fp16 + denom fp16) stream out per chunk on the
gpsimd DMA ring; the host divides and transposes.  The audio phase shares
PSUM with the main loop; inter is flattened to a row-major [1, 12288]
single-partition row via SBUF->SBUF DMAs.  Dummy matmul bursts keep the PE
clock gate warm through the startup DMA fill.
"""

import numpy as np

# Problem constants (hardcoded per harness contract).
B, T = 32, 64
ASIZE, VSIZE, HSIZE, MSIZE = 128, 512, 512, 48
NCORES = 8
BT = B * T                     # 2048
PER = BT // NCORES             # 256 samples per core
R = PER * MSIZE                # 12288 video rows per core
SUPER = 3072                   # rows per superblock (64 groups of 48)
NSB = R // SUPER               # 4 superblocks
SUB = 512                      # matmul moving-dim block (PSUM bank limit)
DSUB = 2 * SUB                 # 1024-col double block for content/score
NPAIR = R // DSUB              # 12 double blocks
PPS = SUPER // DSUB            # 3 double blocks per superblock
GPS = SUPER // MSIZE           # 64 sample groups per superblock
FGRP = 16                      # groups per finalize chunk

_cached = {}


def _build_nc():
    import concourse.bacc as bacc
    import concourse.mybir as mybir
    import concourse.tile as tile

    f32 = mybir.dt.float32
    f16 = mybir.dt.float16
    AF = mybir.ActivationFunctionType
    AX = mybir.AxisListType

    nc = bacc.Bacc(
        "TRN2",
        target_bir_lowering=False,
        debug=False,
        enable_asserts=False,
        num_devices=NCORES,
    )

    # ---- DRAM I/O ----
    vT_d = nc.dram_tensor("vT", [VSIZE, R], f16, kind="ExternalInput").ap()
    audioT_d = nc.dram_tensor("audioT", [ASIZE, PER], f16, kind="ExternalInput").ap()
    wvideoT_d = nc.dram_tensor("WvideoT", [128, VSIZE // 128, HSIZE], f16, kind="ExternalInput").ap()
    waudioT_d = nc.dram_tensor("WaudioT", [ASIZE, HSIZE], f16, kind="ExternalInput").ap()
    wgT_d = nc.dram_tensor("WgT", [128, HSIZE // 128, MSIZE], f16, kind="ExternalInput").ap()
    wvT_d = nc.dram_tensor("WvT", [128, HSIZE // 128, MSIZE], f16, kind="ExternalInput").ap()
    whT_d = nc.dram_tensor("WhT", [112, 1], f32, kind="ExternalInput").ap()
    bvideo_d = nc.dram_tensor("b_video", [128, HSIZE // 128], f32, kind="ExternalInput").ap()
    baudio_d = nc.dram_tensor("b_audio", [128, HSIZE // 128], f32, kind="ExternalInput").ap()
    cT_d = nc.dram_tensor("cT", [VSIZE, PER], f16, kind="ExternalOutput").ap()
    # ez row 0 per sample-column; the host computes denom = group-sums of 48
    # in fp32 (cheaper and more accurate than on-device fp16 reduces)
    ezrow_d = nc.dram_tensor("ezrow", [1, R], f16, kind="ExternalOutput").ap()

    KC = VSIZE // 128          # 4 contraction chunks for the main matmul
    HC = HSIZE // 128          # 4 h chunks

    with tile.TileContext(nc) as tc:
        with (
            tc.tile_pool(name="const", bufs=1) as const,
        ):
            # ---- constants / weights.  Audio-path tensors go on the scalar
            # ring (they gate the first PE work); the big main-loop weights go
            # on the sync ring after the first video chunk so the ACT engine
            # never pays their DMA-issue cost. ----
            audioT_sb = const.tile([128, PER], f16)
            nc.scalar.dma_start(out=audioT_sb, in_=audioT_d)
            waudioT_sb = const.tile([128, HSIZE], f16)
            nc.scalar.dma_start(out=waudioT_sb, in_=waudioT_d)
            baudio_sb = const.tile([128, HC], f32)
            nc.scalar.dma_start(out=baudio_sb, in_=baudio_d)
            wgT_sb = const.tile([128, HC, MSIZE], f16)
            nc.scalar.dma_start(out=wgT_sb, in_=wgT_d)
            # main-loop weights ride the gpsimd ring (idle early), keeping the
            # sync ring free so the first video chunks issue immediately;
            # wvideoT first -- it gates the first mains matmul
            wvideoT_sb = const.tile([128, KC, HSIZE], f16)
            nc.gpsimd.dma_start(out=wvideoT_sb, in_=wvideoT_d)
            bvideo_sb = const.tile([128, HC], f32)
            nc.gpsimd.dma_start(out=bvideo_sb, in_=bvideo_d)
            wvT_sb = const.tile([128, HC, MSIZE], f16)
            nc.gpsimd.dma_start(out=wvT_sb, in_=wvT_d)
            whT_sb = const.tile([112, 1], f32)
            nc.gpsimd.dma_start(out=whT_sb, in_=whT_d)
            ones_m = const.tile([112, 128], f32)
            nc.vector.memset(ones_m, 1.0)
            # W_h replicated across 128 free cols, on partitions 0-47 AND
            # 64-111 (rows 48-63 zero) for the two row-tiled z matmuls
            whB_sb = const.tile([112, 128], f16)
            nc.scalar.mul(out=whB_sb, in_=ones_m, mul=whT_sb)
            # HAM warm-up: keep the PE busy during the initial DMA fill so the
            # clock gate is at 8/8 (2.4 GHz) before the real matmuls arrive
            warm_sb = const.tile([128, 64], f16)
            nc.vector.memset(warm_sb.bitcast(f32), 0.0)
            ones_f32 = const.tile([1, 128], f32)
            nc.vector.memset(ones_f32, 1.0)
            ones48 = const.tile([1, MSIZE], f16)
            nc.vector.tensor_copy(out=ones48, in_=ones_f32[:, :MSIZE])

            # persistent accumulators
            cT_acc = const.tile([128, KC, PER], f16)
            interflat_all = const.tile([1, R], f16)

            with (
                tc.tile_pool(name="vt", bufs=3) as vtp,
                tc.tile_pool(name="vrelu", bufs=2) as vrp,
                tc.tile_pool(name="tanhp", bufs=2) as thp,
                tc.tile_pool(name="ezb", bufs=2) as ezp,
                tc.tile_pool(name="tree", bufs=2) as trp,
                tc.tile_pool(name="mm_ps", bufs=2, space="PSUM") as mm_ps,
                tc.tile_pool(name="ct_ps", bufs=1, space="PSUM") as ct_ps,
                tc.tile_pool(name="z_ps", bufs=1, space="PSUM") as z_ps,
            ):
                warm_ps = mm_ps.tile([64, 64], f32, tag="v_ps", name="warm_ps")

                def warm_burst(n):
                    for _ in range(n):
                        nc.tensor.matmul(
                            warm_ps, warm_sb[:, :64], warm_sb, start=True, stop=True
                        )

                warm_burst(75)

                def emit_audio():
                    # a.T = relu(W_audio.T^T @ audio.T + b_audio); runs on the
                    # PE right after the first mains pair (its DMAs land much
                    # earlier than the video stream)
                    aT_sb = const.tile([128, HC, PER], f16)
                    for m in range(HC):
                        a_ps = mm_ps.tile([128, PER], f32, tag="v_ps",
                                          name=f"a_ps_{m}")
                        nc.tensor.matmul(
                            a_ps,
                            waudioT_sb[:, m * 128 : (m + 1) * 128],
                            audioT_sb,
                            start=True,
                            stop=True,
                        )
                        nc.scalar.activation(
                            out=aT_sb[:, m, :], in_=a_ps, func=AF.Relu,
                            bias=baudio_sb[:, m : m + 1],
                        )
                    # inter[bt, m] = a @ W_g.T, natural layout for a flat write
                    inter_sb = const.tile([128, PER // 128, MSIZE], f16)
                    for t in range(PER // 128):
                        i_ps = mm_ps.tile([128, MSIZE], f32, tag="v_ps",
                                          name=f"i_ps_{t}")
                        for k in range(HC):
                            nc.tensor.matmul(
                                i_ps,
                                aT_sb[:, k, t * 128 : (t + 1) * 128],
                                wgT_sb[:, k, :],
                                start=(k == 0),
                                stop=(k == HC - 1),
                            )
                        nc.scalar.copy(out=inter_sb[:, t, :], in_=i_ps)
                    # flatten inter [bt, m] row-major into a single-partition
                    # row via SBUF->SBUF DMA (no HBM roundtrip)
                    for t in range(PER // 128):
                        nc.gpsimd.dma_start(
                            out=interflat_all[
                                :, t * 128 * MSIZE : (t + 1) * 128 * MSIZE
                            ],
                            in_=inter_sb[:, t, :],
                        )

                vt_t, vr_t, th_t, ez_t = {}, {}, {}, {}

                def emit_mains(q, ms):
                    sb, p = divmod(q, PPS)
                    if p == 0 and ms[0] == 0:
                        vt_t[sb] = vtp.tile([128, KC, SUPER], f16, tag="vt",
                                            name=f"vt_{sb}")
                        if sb == 0:
                            # first double-block arrives per k-chunk (2 KB
                            # contiguous lines, small transfers) so the very
                            # first matmul can start ~10.5us; the rest in
                            # 1024-col chunks
                            for k in range(KC):
                                nc.sync.dma_start(
                                    out=vt_t[sb][:, k, 0:DSUB],
                                    in_=vT_d[k * 128 : (k + 1) * 128, 0:DSUB],
                                )
                            # later chunks ride the scalar ring (free after
                            # the audio weights, and ACT has no work yet) so
                            # two DMA queues drain in parallel during startup
                            for cc in range(1, PPS):
                                nc.scalar.dma_start(
                                    out=vt_t[sb][:, :, cc * DSUB : (cc + 1) * DSUB],
                                    in_=vT_d[
                                        :, cc * DSUB : (cc + 1) * DSUB
                                    ].rearrange("(c p) n -> p c n", p=128),
                                )
                        else:
                            nc.sync.dma_start(
                                out=vt_t[sb],
                                in_=vT_d[:, sb * SUPER : (sb + 1) * SUPER].rearrange(
                                    "(c p) n -> p c n", p=128
                                ),
                            )
                        vr_t[sb] = vrp.tile([128, HC, SUPER], f16, tag="vrelu",
                                            name=f"vrelu_{sb}")
                        # tanh halves: col half A on partitions 0-47, half B on
                        # 64-111, both at free offset 512p (same ACT op)
                        th_t[sb] = thp.tile([112, SUPER // 2], f16, tag="tanhc",
                                            name=f"tanhc_{sb}")
                        ez_t[sb] = ezp.tile([128, SUPER], f16, tag="ezb",
                                            name=f"ezb_{sb}")
                    c0 = p * DSUB
                    for m in ms:
                        # both 512-col halves of this m-chunk accumulate into
                        # one 2-bank PSUM tile -> ONE fused relu of 1024 cols
                        v_ps = mm_ps.tile([128, 2, SUB], f32, tag="v_ps",
                                          name=f"v_ps_{q}_{m}")
                        for j in range(2):
                            for k in range(KC):
                                nc.tensor.matmul(
                                    v_ps[:, j, :],
                                    wvideoT_sb[:, k, m * 128 : (m + 1) * 128],
                                    vt_t[sb][:, k, c0 + j * SUB : c0 + (j + 1) * SUB],
                                    start=(k == 0),
                                    stop=(k == KC - 1),
                                )
                        nc.scalar.activation(
                            out=vr_t[sb][:, m, c0 : c0 + DSUB].rearrange(
                                "p (a b) -> p a b", a=2
                            ),
                            in_=v_ps, func=AF.Relu, bias=bvideo_sb[:, m : m + 1],
                        )

                def emit_content(q):
                    # two 512-col half chains packed onto array col groups
                    # 0-47 / 64-111; the 5-matmul accumulation chains run
                    # concurrently on the PE
                    sb, p = divmod(q, PPS)
                    c0 = p * DSUB
                    r0 = sb * SUPER
                    ct = ct_ps.tile([128, SUB], f32, tag="ct", name=f"ct_{q}")
                    ctA = ct[0:MSIZE, :]
                    ctB = ct[64 : 64 + MSIZE, :]
                    nc.tensor.matmul(
                        ctA, ones48,
                        interflat_all[:, r0 + c0 : r0 + c0 + SUB],
                        start=True, stop=False,
                    )
                    nc.tensor.matmul(
                        ctB, ones48,
                        interflat_all[:, r0 + c0 + SUB : r0 + c0 + DSUB],
                        start=True, stop=False,
                    )
                    for k in range(HC):
                        nc.tensor.matmul(
                            ctA, wvT_sb[:, k, :], vr_t[sb][:, k, c0 : c0 + SUB],
                            start=False, stop=(k == HC - 1),
                        )
                        nc.tensor.matmul(
                            ctB, wvT_sb[:, k, :],
                            vr_t[sb][:, k, c0 + SUB : c0 + DSUB],
                            start=False, stop=(k == HC - 1),
                        )
                    # one fused tanh over partitions 0-111 (rows 48-63 junk)
                    nc.scalar.activation(
                        out=th_t[sb][0:112, p * SUB : (p + 1) * SUB],
                        in_=ct[0:112, :], func=AF.Tanh,
                    )

                def emit_zmm(q):
                    sb, p = divmod(q, PPS)
                    # two row-tiled z matmuls (array rows 0-47 / 64-111) into
                    # the two banks of one PSUM tile, concurrent on the PE
                    zt = z_ps.tile([128, 2, SUB], f32, tag="zt", name=f"z_{q}")
                    nc.tensor.matmul(
                        zt[:, 0, :], whB_sb[0:MSIZE, :],
                        th_t[sb][0:MSIZE, p * SUB : (p + 1) * SUB],
                        start=True, stop=True,
                    )
                    nc.tensor.matmul(
                        zt[:, 1, :], whB_sb[64 : 64 + MSIZE, :],
                        th_t[sb][64 : 64 + MSIZE, p * SUB : (p + 1) * SUB],
                        start=True, stop=True,
                    )
                    return zt

                def emit_expmul(q, zt, j=None):
                    # exp + in-place weighted multiply; j=None does the full
                    # DSUB in one fused exp + one mul, j=0/1 does one half
                    sb, p = divmod(q, PPS)
                    c0 = p * DSUB
                    if j is None:
                        nc.scalar.activation(
                            out=ez_t[sb][:, c0 : c0 + DSUB].rearrange(
                                "p (a b) -> p a b", a=2
                            ),
                            in_=zt, func=AF.Exp,
                        )
                        nc.vector.tensor_mul(
                            vt_t[sb][:, :, c0 : c0 + DSUB],
                            vt_t[sb][:, :, c0 : c0 + DSUB],
                            ez_t[sb][:, c0 : c0 + DSUB]
                            .unsqueeze(1)
                            .broadcast_to([128, KC, DSUB]),
                        )
                    else:
                        cj = c0 + j * SUB
                        nc.scalar.activation(
                            out=ez_t[sb][:, cj : cj + SUB], in_=zt[:, j, :],
                            func=AF.Exp,
                        )
                        nc.vector.tensor_mul(
                            vt_t[sb][:, :, cj : cj + SUB],
                            vt_t[sb][:, :, cj : cj + SUB],
                            ez_t[sb][:, cj : cj + SUB]
                            .unsqueeze(1)
                            .broadcast_to([128, KC, SUB]),
                        )

                def emit_score(q):
                    emit_expmul(q, emit_zmm(q))

                def emit_finalize(sb, g0f, ng, dma=None):
                    # reduce cols [g0f*48, (g0f+ng)*48) of superblock sb
                    # (units of 48-col sample groups, local to sb).
                    # dma=(gd0, ngd) flushes that global group range of
                    # cT_acc to DRAM.
                    ch = g0f * MSIZE
                    g0 = sb * GPS + g0f
                    lp = nc.allow_low_precision(
                        reason="fp16 group sums; fp32 internal accum"
                    )
                    lp.__enter__()
                    tree = trp.tile([128, KC, FGRP, MSIZE // 2], f16, tag="tree",
                                    name=f"tree_{sb}_{g0f}")
                    wv = vt_t[sb][:, :, ch : ch + ng * MSIZE].rearrange(
                        "p c (g n) -> p c g n", n=MSIZE
                    )
                    tr = tree[:, :, :ng, :]
                    nc.vector.tensor_add(
                        tr, wv[:, :, :, : MSIZE // 2], wv[:, :, :, MSIZE // 2 :]
                    )
                    nc.vector.tensor_add(
                        tr[:, :, :, : MSIZE // 4],
                        tr[:, :, :, : MSIZE // 4],
                        tr[:, :, :, MSIZE // 4 :],
                    )
                    nc.vector.tensor_add(
                        tr[:, :, :, : MSIZE // 8],
                        tr[:, :, :, : MSIZE // 8],
                        tr[:, :, :, MSIZE // 8 : MSIZE // 4],
                    )
                    nc.vector.reduce_sum(
                        out=cT_acc[:, :, g0 : g0 + ng],
                        in_=tr[:, :, :, : MSIZE // 8],
                        axis=AX.X,
                    )
                    lp.__exit__(None, None, None)
                    if dma is not None:
                        gd0, ngd = dma
                        nc.gpsimd.dma_start(
                            out=cT_d[:, gd0 : gd0 + ngd].rearrange(
                                "(c p) n -> p c n", p=128
                            ),
                            in_=cT_acc[:, :, gd0 : gd0 + ngd],
                        )

                def emit_ezrow(sb):
                    nc.gpsimd.dma_start(
                        out=ezrow_d[:, sb * SUPER : (sb + 1) * SUPER],
                        in_=ez_t[sb][0:1, :],
                    )

                # software-pipelined emission with a ONE-pair lag, interleaved
                # at half-pair granularity so no consumer head-of-line-blocks
                # the PE queue:
                #   mains(q) m0,m1 | content(q-1) | mains(q) m2,m3 |
                #   score(q-1) | finalize chunks of (q-1)
                # content(q-1)'s tanh drains during m2/m3, so the z matmuls
                # never stall; exp runs after m2/m3's relus; the weighted
                # multiply + tree chunks fill DVE one pair behind the mains.
                # Finalize chunks fire as soon as their columns are weighted:
                # after the p-th score of a superblock, columns to 1024(p+1)
                # are done, covering groups up to floor(1024(p+1)/48).  The
                # LAST superblock runs its exp/mul per 512-col half with
                # 8-group chunks chasing each half, so only ~mul+tree for 16
                # groups trails the final matmul.
                lsb = NSB - 1

                def emit_fins(qq):
                    sb2, p2 = divmod(qq, PPS)
                    if p2 == 0:
                        emit_finalize(sb2, 0, FGRP)
                    elif p2 == 1:
                        emit_finalize(sb2, FGRP, FGRP)
                        emit_finalize(sb2, 2 * FGRP, 10)
                    else:
                        emit_finalize(sb2, 42, 11)
                        emit_finalize(sb2, 53, 11, dma=(sb2 * GPS, GPS))
                        emit_ezrow(sb2)

                def emit_consume(qq):
                    sb2, p2 = divmod(qq, PPS)
                    if sb2 < lsb:
                        emit_score(qq)
                        emit_fins(qq)
                        return
                    # last superblock: exp/mul per 512-col half, with the
                    # largest group-aligned chunk each half unlocks
                    # (halves end at 512(2p+j+1); chunk ends at 48(g0+ng))
                    zt = emit_zmm(qq)
                    emit_expmul(qq, zt, j=0)
                    if p2 == 0:
                        emit_finalize(lsb, 0, 10)
                    elif p2 == 1:
                        emit_finalize(lsb, 21, 11)
                    else:
                        emit_finalize(lsb, 42, 11, dma=(lsb * GPS + 42, 11))
                    emit_expmul(qq, zt, j=1)
                    if p2 == 0:
                        emit_finalize(lsb, 10, 11)
                    elif p2 == 1:
                        emit_finalize(lsb, 32, 10, dma=(lsb * GPS, 42))
                    else:
                        emit_finalize(lsb, 53, 11, dma=(lsb * GPS + 53, 11))
                        emit_ezrow(lsb)

                emit_mains(0, (0, 1))
                emit_mains(0, (2, 3))
                emit_audio()
                for q in range(1, NPAIR):
                    emit_mains(q, (0, 1))
                    emit_content(q - 1)
                    # consume between m2 and m3: content(q-1)'s tanh drains
                    # during m2, so the z matmuls slot in without stalling
                    # and the DVE work starts ~1.7us earlier
                    emit_mains(q, (2,))
                    emit_consume(q - 1)
                    emit_mains(q, (3,))
                emit_content(NPAIR - 1)
                emit_consume(NPAIR - 1)

    nc.compile()
    return nc


def _prep_in_maps(inputs):
    audio = np.ascontiguousarray(np.asarray(inputs["audio"], np.float32))
    video = np.ascontiguousarray(np.asarray(inputs["video"], np.float32))
    def dev_chunks(w):  # [C*128, X] -> [128, C, X] (partition-major chunks)
        a = np.asarray(w)
        return np.ascontiguousarray(a.reshape(-1, 128, a.shape[-1]).transpose(1, 0, 2))

    WvideoT = dev_chunks(np.asarray(inputs["W_video"], np.float32).T.astype(np.float16))
    WaudioT = np.ascontiguousarray(np.asarray(inputs["W_audio"], np.float32).T.astype(np.float16))
    WgT = dev_chunks(np.asarray(inputs["W_g"], np.float32).T.astype(np.float16))
    WvT = dev_chunks(np.asarray(inputs["W_v"], np.float32).T.astype(np.float16))
    wh = np.asarray(inputs["W_h"], np.float32).T  # [48, 1]
    WhT = np.zeros((112, 1), np.float32)
    WhT[0:MSIZE] = wh
    WhT[64 : 64 + MSIZE] = wh
    WhT = np.ascontiguousarray(WhT)
    b_video = np.ascontiguousarray(
        np.asarray(inputs["b_video"], np.float32).reshape(-1, 128).T
    )
    b_audio = np.ascontiguousarray(
        np.asarray(inputs["b_audio"], np.float32).reshape(-1, 128).T
    )

    a2 = audio.reshape(BT, ASIZE).astype(np.float16)
    v2 = video.reshape(BT, MSIZE, VSIZE).astype(np.float16)
    in_maps = []
    for c in range(NCORES):
        sl = slice(c * PER, (c + 1) * PER)
        vT = np.ascontiguousarray(v2[sl].reshape(R, VSIZE).T)
        audioT = np.ascontiguousarray(a2[sl].T)
        in_maps.append(
            {
                "vT": vT,
                "audioT": audioT,
                "WvideoT": WvideoT,
                "WaudioT": WaudioT,
                "WgT": WgT,
                "WvT": WvT,
                "WhT": WhT,
                "b_video": b_video,
                "b_audio": b_audio,
            }
        )
    return in_maps


def _run(inputs, trace=False, **spmd_kwargs):
    from concourse.bass_utils import run_bass_kernel_spmd

    if "nc" not in _cached:
        _cached["nc"] = _build_nc()
    nc = _cached["nc"]
    in_maps = _prep_in_maps(inputs)
    res = run_bass_kernel_spmd(
        nc, in_maps, core_ids=list(range(NCORES)), trace=trace, **spmd_kwargs
    )
    def _part(r):
        denom = r["ezrow"].astype(np.float32).reshape(PER, MSIZE).sum(axis=1)
        return (r["cT"].astype(np.float32) / denom[None, :]).T

    parts = [_part(r) for r in res.results]
    out = np.concatenate(parts, axis=0).reshape(B, T, VSIZE)
    return np.ascontiguousarray(out.astype(np.float32)), res


def kernel(**inputs):
    out, _ = _run(inputs, trace=False)
    return out


# revision 32
# speedup vs baseline: 1.0266x; 1.0266x over previous
"""Trainium2 Bass kernel for the audio-visual attention model.

Math (per (b,t) sample, BT = 32*64 = 2048 of them):
    V   = video[b,t]                              # [48, 512]
    v   = relu(V @ W_video.T + b_video)           # [48, 512]
    a   = relu(audio[b,t] @ W_audio.T + b_audio)  # [512]
    inter   = a @ W_g.T                           # [48]
    content = v @ W_v.T + inter[:, None]          # [48, 48]
    z   = tanh(content) @ W_h.T                   # [48]
    alpha = softmax(z)
    out = alpha @ V                               # [512]

Strategy: data-parallel over BT across 8 cores (256 samples each, R = 256*48
= 12288 video rows per core).  The host pre-transposes the video shard to
V.T [512, 12288], pre-arranges weights into device layouts, and runs the
matmul chain in fp16 (~7e-4 rel err, 1 row/cycle on the PE).

Per-core pipeline over 4 superblocks of 3072 rows, processed in 12
double-sub-blocks (DSUB=1024 cols, two 512-col halves):
    vT.relu   relu(W_video.T^T @ V.T + b_video)      PE + ACT
              -- per m-chunk, the two 512-col halves accumulate into one
                 2-bank PSUM tile; ONE fused relu covers 1024 cols
    content.T W_v.T^T @ vT.relu + ones^T @ inter     PE [48, cols]
              -- col half A -> array col group 0-47, half B -> 64-111
                 (tile_position col tiling): the two 5-matmul chains run
                 CONCURRENTLY, halving content PE time
    tanh      ONE fused op over partitions 0-111: half A lands in
              th[0:48, 512p:+512], half B in th[64:112, 512p:+512]
              (rows 48-63 hold junk, never read)
    z         two row-tiled matmuls (rows 0-47 / 64-111) -> one 2-bank
              PSUM tile, CONCURRENT on the PE                PE [128, cols]
    ez=exp(z) ONE fused op per DSUB                          ACT -> fp16
    weighted  V.T * ez in place                              DVE 2x mode
    cT groups halving-tree adds + reduce per 48-col group    DVE
              -- emitted in <=16-group chunks as soon as the covering
                 columns are multiplied; the last superblock's chunks are
                 staged (10/11/11 groups) so only ~2 small chunks trail
                 the final matmul, each gated on a half-DSUB mul
    denom     48-group sums of ez row 0                      DVE (per chunk)
Outputs (unnormalized cT fp16 + denom fp16) stream out per chunk on the
gpsimd DMA ring; the host divides and transposes.  The audio phase shares
PSUM with the main loop; inter is flattened to a row-major [1, 12288]
single-partition row via SBUF->SBUF DMAs.  Dummy matmul bursts keep the PE
clock gate warm through the startup DMA fill.
"""

import numpy as np

# Problem constants (hardcoded per harness contract).
B, T = 32, 64
ASIZE, VSIZE, HSIZE, MSIZE = 128, 512, 512, 48
NCORES = 8
BT = B * T                     # 2048
PER = BT // NCORES             # 256 samples per core
R = PER * MSIZE                # 12288 video rows per core
SUPER = 3072                   # rows per superblock (64 groups of 48)
NSB = R // SUPER               # 4 superblocks
SUB = 512                      # matmul moving-dim block (PSUM bank limit)
DSUB = 2 * SUB                 # 1024-col double block for content/score
NPAIR = R // DSUB              # 12 double blocks
PPS = SUPER // DSUB            # 3 double blocks per superblock
GPS = SUPER // MSIZE           # 64 sample groups per superblock
FGRP = 16                      # groups per finalize chunk

_cached = {}


def _build_nc():
    import concourse.bacc as bacc
    import concourse.mybir as mybir
    import concourse.tile as tile

    f32 = mybir.dt.float32
    f16 = mybir.dt.float16
    AF = mybir.ActivationFunctionType
    AX = mybir.AxisListType

    nc = bacc.Bacc(
        "TRN2",
        target_bir_lowering=False,
        debug=False,
        enable_asserts=False,
        num_devices=NCORES,
    )

    # ---- DRAM I/O ----
    vT_d = nc.dram_tensor("vT", [VSIZE, R], f16, kind="ExternalInput").ap()
    audioT_d = nc.dram_tensor("audioT", [ASIZE, PER], f16, kind="ExternalInput").ap()
    wvideoT_d = nc.dram_tensor("WvideoT", [128, VSIZE // 128, HSIZE], f16, kind="ExternalInput").ap()
    waudioT_d = nc.dram_tensor("WaudioT", [ASIZE, HSIZE], f16, kind="ExternalInput").ap()
    wgT_d = nc.dram_tensor("WgT", [128, HSIZE // 128, MSIZE], f16, kind="ExternalInput").ap()
    wvT_d = nc.dram_tensor("WvT", [128, HSIZE // 128, MSIZE], f16, kind="ExternalInput").ap()
    whT_d = nc.dram_tensor("WhT", [112, 1], f32, kind="ExternalInput").ap()
    bvideo_d = nc.dram_tensor("b_video", [128, HSIZE // 128], f32, kind="ExternalInput").ap()
    baudio_d = nc.dram_tensor("b_audio", [128, HSIZE // 128], f32, kind="ExternalInput").ap()
    cT_d = nc.dram_tensor("cT", [VSIZE, PER], f16, kind="ExternalOutput").ap()
    # ez row 0 per sample-column; the host computes denom = group-sums of 48
    # in fp32 (cheaper and more accurate than on-device fp16 reduces)
    ezrow_d = nc.dram_tensor("ezrow", [1, R], f16, kind="ExternalOutput").ap()

    KC = VSIZE // 128          # 4 contraction chunks for the main matmul
    HC = HSIZE // 128          # 4 h chunks

    with tile.TileContext(nc) as tc:
        with (
            tc.tile_pool(name="const", bufs=1) as const,
        ):
            # ---- constants / weights.  Audio-path tensors go on the scalar
            # ring (they gate the first PE work); the big main-loop weights go
            # on the sync ring after the first video chunk so the ACT engine
            # never pays their DMA-issue cost. ----
            audioT_sb = const.tile([128, PER], f16)
            nc.scalar.dma_start(out=audioT_sb, in_=audioT_d)
            waudioT_sb = const.tile([128, HSIZE], f16)
            nc.scalar.dma_start(out=waudioT_sb, in_=waudioT_d)
            baudio_sb = const.tile([128, HC], f32)
            nc.scalar.dma_start(out=baudio_sb, in_=baudio_d)
            wgT_sb = const.tile([128, HC, MSIZE], f16)
            nc.scalar.dma_start(out=wgT_sb, in_=wgT_d)
            # main-loop weights ride the gpsimd ring (idle early), keeping the
            # sync ring free so the first video chunks issue immediately;
            # wvideoT first -- it gates the first mains matmul
            wvideoT_sb = const.tile([128, KC, HSIZE], f16)
            nc.gpsimd.dma_start(out=wvideoT_sb, in_=wvideoT_d)
            bvideo_sb = const.tile([128, HC], f32)
            nc.gpsimd.dma_start(out=bvideo_sb, in_=bvideo_d)
            wvT_sb = const.tile([128, HC, MSIZE], f16)
            nc.gpsimd.dma_start(out=wvT_sb, in_=wvT_d)
            whT_sb = const.tile([112, 1], f32)
            nc.gpsimd.dma_start(out=whT_sb, in_=whT_d)
            ones_m = const.tile([112, 128], f32)
            nc.vector.memset(ones_m, 1.0)
            # W_h replicated across 128 free cols, on partitions 0-47 AND
            # 64-111 (rows 48-63 zero) for the two row-tiled z matmuls
            whB_sb = const.tile([112, 128], f16)
            nc.scalar.mul(out=whB_sb, in_=ones_m, mul=whT_sb)
            # HAM warm-up: keep the PE busy during the initial DMA fill so the
            # clock gate is at 8/8 (2.4 GHz) before the real matmuls arrive
            warm_sb = const.tile([128, 64], f16)
            nc.vector.memset(warm_sb.bitcast(f32), 0.0)
            ones_f32 = const.tile([1, 128], f32)
            nc.vector.memset(ones_f32, 1.0)
            ones48 = const.tile([1, MSIZE], f16)
            nc.vector.tensor_copy(out=ones48, in_=ones_f32[:, :MSIZE])

            # persistent accumulators
            cT_acc = const.tile([128, KC, PER], f16)
            interflat_all = const.tile([1, R], f16)

            with (
                tc.tile_pool(name="vt", bufs=3) as vtp,
                tc.tile_pool(name="vrelu", bufs=2) as vrp,
                tc.tile_pool(name="tanhp", bufs=2) as thp,
                tc.tile_pool(name="ezb", bufs=2) as ezp,
                tc.tile_pool(name="tree", bufs=2) as trp,
                tc.tile_pool(name="mm_ps", bufs=2, space="PSUM") as mm_ps,
                tc.tile_pool(name="ct_ps", bufs=1, space="PSUM") as ct_ps,
                tc.tile_pool(name="z_ps", bufs=1, space="PSUM") as z_ps,
            ):
                warm_ps = mm_ps.tile([64, 64], f32, tag="v_ps", name="warm_ps")

                def warm_burst(n):
                    for _ in range(n):
                        nc.tensor.matmul(
                            warm_ps, warm_sb[:, :64], warm_sb, start=True, stop=True
                        )

                warm_burst(105)

                def emit_audio():
                    # a.T = relu(W_audio.T^T @ audio.T + b_audio); runs on the
                    # PE right after the first mains pair (its DMAs land much
                    # earlier than the video stream)
                    aT_sb = const.tile([128, HC, PER], f16)
                    for m in range(HC):
                        a_ps = mm_ps.tile([128, PER], f32, tag="v_ps",
                                          name=f"a_ps_{m}")
                        nc.tensor.matmul(
                            a_ps,
                            waudioT_sb[:, m * 128 : (m + 1) * 128],
                            audioT_sb,
                            start=True,
                            stop=True,
                        )
                        nc.scalar.activation(
                            out=aT_sb[:, m, :], in_=a_ps, func=AF.Relu,
                            bias=baudio_sb[:, m : m + 1],
                        )
                    # inter[bt, m] = a @ W_g.T, natural layout for a flat write
                    inter_sb = const.tile([128, PER // 128, MSIZE], f16)
                    for t in range(PER // 128):
                        i_ps = mm_ps.tile([128, MSIZE], f32, tag="v_ps",
                                          name=f"i_ps_{t}")
                        for k in range(HC):
                            nc.tensor.matmul(
                                i_ps,
                                aT_sb[:, k, t * 128 : (t + 1) * 128],
                                wgT_sb[:, k, :],
                                start=(k == 0),
                                stop=(k == HC - 1),
                            )
                        nc.scalar.copy(out=inter_sb[:, t, :], in_=i_ps)
                    # flatten inter [bt, m] row-major into a single-partition
                    # row via SBUF->SBUF DMA (no HBM roundtrip)
                    for t in range(PER // 128):
                        nc.gpsimd.dma_start(
                            out=interflat_all[
                                :, t * 128 * MSIZE : (t + 1) * 128 * MSIZE
                            ],
                            in_=inter_sb[:, t, :],
                        )

                vt_t, vr_t, th_t, ez_t = {}, {}, {}, {}

                def emit_mains(q, ms):
                    sb, p = divmod(q, PPS)
                    if p == 0 and ms[0] == 0:
                        vt_t[sb] = vtp.tile([128, KC, SUPER], f16, tag="vt",
                                            name=f"vt_{sb}")
                        if sb == 0:
                            # first double-block arrives per k-chunk (2 KB
                            # contiguous lines, small transfers) so the very
                            # first matmul can start ~10.5us; the rest in
                            # 1024-col chunks
                            for k in range(KC):
                                nc.sync.dma_start(
                                    out=vt_t[sb][:, k, 0:DSUB],
                                    in_=vT_d[k * 128 : (k + 1) * 128, 0:DSUB],
                                )
                            for cc in range(1, PPS):
                                nc.sync.dma_start(
                                    out=vt_t[sb][:, :, cc * DSUB : (cc + 1) * DSUB],
                                    in_=vT_d[
                                        :, cc * DSUB : (cc + 1) * DSUB
                                    ].rearrange("(c p) n -> p c n", p=128),
                                )
                        else:
                            nc.sync.dma_start(
                                out=vt_t[sb],
                                in_=vT_d[:, sb * SUPER : (sb + 1) * SUPER].rearrange(
                                    "(c p) n -> p c n", p=128
                                ),
                            )
                        vr_t[sb] = vrp.tile([128, HC, SUPER], f16, tag="vrelu",
                                            name=f"vrelu_{sb}")
                        # tanh halves: col half A on partitions 0-47, half B on
                        # 64-111, both at free offset 512p (same ACT op)
                        th_t[sb] = thp.tile([112, SUPER // 2], f16, tag="tanhc",
                                            name=f"tanhc_{sb}")
                        ez_t[sb] = ezp.tile([128, SUPER], f16, tag="ezb",
                                            name=f"ezb_{sb}")
                    c0 = p * DSUB
                    for m in ms:
                        # both 512-col halves of this m-chunk accumulate into
                        # one 2-bank PSUM tile -> ONE fused relu of 1024 cols
                        v_ps = mm_ps.tile([128, 2, SUB], f32, tag="v_ps",
                                          name=f"v_ps_{q}_{m}")
                        for j in range(2):
                            for k in range(KC):
                                nc.tensor.matmul(
                                    v_ps[:, j, :],
                                    wvideoT_sb[:, k, m * 128 : (m + 1) * 128],
                                    vt_t[sb][:, k, c0 + j * SUB : c0 + (j + 1) * SUB],
                                    start=(k == 0),
                                    stop=(k == KC - 1),
                                )
                        nc.scalar.activation(
                            out=vr_t[sb][:, m, c0 : c0 + DSUB].rearrange(
                                "p (a b) -> p a b", a=2
                            ),
                            in_=v_ps, func=AF.Relu, bias=bvideo_sb[:, m : m + 1],
                        )

                def emit_content(q):
                    # two 512-col half chains packed onto array col groups
                    # 0-47 / 64-111; the 5-matmul accumulation chains run
                    # concurrently on the PE
                    sb, p = divmod(q, PPS)
                    c0 = p * DSUB
                    r0 = sb * SUPER
                    ct = ct_ps.tile([128, SUB], f32, tag="ct", name=f"ct_{q}")
                    ctA = ct[0:MSIZE, :]
                    ctB = ct[64 : 64 + MSIZE, :]
                    nc.tensor.matmul(
                        ctA, ones48,
                        interflat_all[:, r0 + c0 : r0 + c0 + SUB],
                        start=True, stop=False,
                    )
                    nc.tensor.matmul(
                        ctB, ones48,
                        interflat_all[:, r0 + c0 + SUB : r0 + c0 + DSUB],
                        start=True, stop=False,
                    )
                    for k in range(HC):
                        nc.tensor.matmul(
                            ctA, wvT_sb[:, k, :], vr_t[sb][:, k, c0 : c0 + SUB],
                            start=False, stop=(k == HC - 1),
                        )
                        nc.tensor.matmul(
                            ctB, wvT_sb[:, k, :],
                            vr_t[sb][:, k, c0 + SUB : c0 + DSUB],
                            start=False, stop=(k == HC - 1),
                        )
                    # one fused tanh over partitions 0-111 (rows 48-63 junk)
                    nc.scalar.activation(
                        out=th_t[sb][0:112, p * SUB : (p + 1) * SUB],
                        in_=ct[0:112, :], func=AF.Tanh,
                    )

                def emit_zmm(q):
                    sb, p = divmod(q, PPS)
                    # two row-tiled z matmuls (array rows 0-47 / 64-111) into
                    # the two banks of one PSUM tile, concurrent on the PE
                    zt = z_ps.tile([128, 2, SUB], f32, tag="zt", name=f"z_{q}")
                    nc.tensor.matmul(
                        zt[:, 0, :], whB_sb[0:MSIZE, :],
                        th_t[sb][0:MSIZE, p * SUB : (p + 1) * SUB],
                        start=True, stop=True,
                    )
                    nc.tensor.matmul(
                        zt[:, 1, :], whB_sb[64 : 64 + MSIZE, :],
                        th_t[sb][64 : 64 + MSIZE, p * SUB : (p + 1) * SUB],
                        start=True, stop=True,
                    )
                    return zt

                def emit_expmul(q, zt, j=None):
                    # exp + in-place weighted multiply; j=None does the full
                    # DSUB in one fused exp + one mul, j=0/1 does one half
                    sb, p = divmod(q, PPS)
                    c0 = p * DSUB
                    if j is None:
                        nc.scalar.activation(
                            out=ez_t[sb][:, c0 : c0 + DSUB].rearrange(
                                "p (a b) -> p a b", a=2
                            ),
                            in_=zt, func=AF.Exp,
                        )
                        nc.vector.tensor_mul(
                            vt_t[sb][:, :, c0 : c0 + DSUB],
                            vt_t[sb][:, :, c0 : c0 + DSUB],
                            ez_t[sb][:, c0 : c0 + DSUB]
                            .unsqueeze(1)
                            .broadcast_to([128, KC, DSUB]),
                        )
                    else:
                        cj = c0 + j * SUB
                        nc.scalar.activation(
                            out=ez_t[sb][:, cj : cj + SUB], in_=zt[:, j, :],
                            func=AF.Exp,
                        )
                        nc.vector.tensor_mul(
                            vt_t[sb][:, :, cj : cj + SUB],
                            vt_t[sb][:, :, cj : cj + SUB],
                            ez_t[sb][:, cj : cj + SUB]
                            .unsqueeze(1)
                            .broadcast_to([128, KC, SUB]),
                        )

                def emit_score(q):
                    emit_expmul(q, emit_zmm(q))

                def emit_finalize(sb, g0f, ng, dma=None):
                    # reduce cols [g0f*48, (g0f+ng)*48) of superblock sb
                    # (units of 48-col sample groups, local to sb).
                    # dma=(gd0, ngd) flushes that global group range of
                    # cT_acc to DRAM.
                    ch = g0f * MSIZE
                    g0 = sb * GPS + g0f
                    lp = nc.allow_low_precision(
                        reason="fp16 group sums; fp32 internal accum"
                    )
                    lp.__enter__()
                    tree = trp.tile([128, KC, FGRP, MSIZE // 2], f16, tag="tree",
                                    name=f"tree_{sb}_{g0f}")
                    wv = vt_t[sb][:, :, ch : ch + ng * MSIZE].rearrange(
                        "p c (g n) -> p c g n", n=MSIZE
                    )
                    tr = tree[:, :, :ng, :]
                    nc.vector.tensor_add(
                        tr, wv[:, :, :, : MSIZE // 2], wv[:, :, :, MSIZE // 2 :]
                    )
                    nc.vector.tensor_add(
                        tr[:, :, :, : MSIZE // 4],
                        tr[:, :, :, : MSIZE // 4],
                        tr[:, :, :, MSIZE // 4 :],
                    )
                    nc.vector.tensor_add(
                        tr[:, :, :, : MSIZE // 8],
                        tr[:, :, :, : MSIZE // 8],
                        tr[:, :, :, MSIZE // 8 : MSIZE // 4],
                    )
                    nc.vector.reduce_sum(
                        out=cT_acc[:, :, g0 : g0 + ng],
                        in_=tr[:, :, :, : MSIZE // 8],
                        axis=AX.X,
                    )
                    lp.__exit__(None, None, None)
                    if dma is not None:
                        gd0, ngd = dma
                        nc.gpsimd.dma_start(
                            out=cT_d[:, gd0 : gd0 + ngd].rearrange(
                                "(c p) n -> p c n", p=128
                            ),
                            in_=cT_acc[:, :, gd0 : gd0 + ngd],
                        )

                def emit_ezrow(sb):
                    nc.gpsimd.dma_start(
                        out=ezrow_d[:, sb * SUPER : (sb + 1) * SUPER],
                        in_=ez_t[sb][0:1, :],
                    )

                # software-pipelined emission with a ONE-pair lag, interleaved
                # at half-pair granularity so no consumer head-of-line-blocks
                # the PE queue:
                #   mains(q) m0,m1 | content(q-1) | mains(q) m2,m3 |
                #   score(q-1) | finalize chunks of (q-1)
                # content(q-1)'s tanh drains during m2/m3, so the z matmuls
                # never stall; exp runs after m2/m3's relus; the weighted
                # multiply + tree chunks fill DVE one pair behind the mains.
                # Finalize chunks fire as soon as their columns are weighted:
                # after the p-th score of a superblock, columns to 1024(p+1)
                # are done, covering groups up to floor(1024(p+1)/48).  The
                # LAST superblock runs its exp/mul per 512-col half with
                # 8-group chunks chasing each half, so only ~mul+tree for 16
                # groups trails the final matmul.
                lsb = NSB - 1

                def emit_fins(qq):
                    sb2, p2 = divmod(qq, PPS)
                    if p2 == 0:
                        emit_finalize(sb2, 0, FGRP)
                    elif p2 == 1:
                        emit_finalize(sb2, FGRP, FGRP)
                        emit_finalize(sb2, 2 * FGRP, 10)
                    else:
                        emit_finalize(sb2, 42, 11)
                        emit_finalize(sb2, 53, 11, dma=(sb2 * GPS, GPS))
                        emit_ezrow(sb2)

                def emit_consume(qq):
                    sb2, p2 = divmod(qq, PPS)
                    if sb2 < lsb:
                        emit_score(qq)
                        emit_fins(qq)
                        return
                    # last superblock: exp/mul per 512-col half, with the
                    # largest group-aligned chunk each half unlocks
                    # (halves end at 512(2p+j+1); chunk ends at 48(g0+ng))
                    zt = emit_zmm(qq)
                    emit_expmul(qq, zt, j=0)
                    if p2 == 0:
                        emit_finalize(lsb, 0, 10)
                    elif p2 == 1:
                        emit_finalize(lsb, 21, 11)
                    else:
                        emit_finalize(lsb, 42, 11, dma=(lsb * GPS + 42, 11))
                    emit_expmul(qq, zt, j=1)
                    if p2 == 0:
                        emit_finalize(lsb, 10, 11)
                    elif p2 == 1:
                        emit_finalize(lsb, 32, 10, dma=(lsb * GPS, 42))
                    else:
                        # ezrow first (only needs the exps, long done); the
                        # final finalize is split so the very last cT DMA is
                        # small and issues ~1us earlier
                        emit_ezrow(lsb)
                        emit_finalize(lsb, 53, 6, dma=(lsb * GPS + 53, 6))
                        emit_finalize(lsb, 59, 5, dma=(lsb * GPS + 59, 5))

                emit_mains(0, (0, 1))
                emit_mains(0, (2, 3))
                emit_audio()
                for q in range(1, NPAIR):
                    emit_mains(q, (0, 1))
                    emit_content(q - 1)
                    # consume between m2 and m3: content(q-1)'s tanh drains
                    # during m2, so the z matmuls slot in without stalling
                    # and the DVE work starts ~1.7us earlier
                    emit_mains(q, (2,))
                    emit_consume(q - 1)
                    emit_mains(q, (3,))
                emit_content(NPAIR - 1)
                emit_consume(NPAIR - 1)

    nc.compile()
    return nc


def _prep_in_maps(inputs):
    audio = np.ascontiguousarray(np.asarray(inputs["audio"], np.float32))
    video = np.ascontiguousarray(np.asarray(inputs["video"], np.float32))
    def dev_chunks(w):  # [C*128, X] -> [128, C, X] (partition-major chunks)
        a = np.asarray(w)
        return np.ascontiguousarray(a.reshape(-1, 128, a.shape[-1]).transpose(1, 0, 2))

    WvideoT = dev_chunks(np.asarray(inputs["W_video"], np.float32).T.astype(np.float16))
    WaudioT = np.ascontiguousarray(np.asarray(inputs["W_audio"], np.float32).T.astype(np.float16))
    WgT = dev_chunks(np.asarray(inputs["W_g"], np.float32).T.astype(np.float16))
    WvT = dev_chunks(np.asarray(inputs["W_v"], np.float32).T.astype(np.float16))
    wh = np.asarray(inputs["W_h"], np.float32).T  # [48, 1]
    WhT = np.zeros((112, 1), np.float32)
    WhT[0:MSIZE] = wh
    WhT[64 : 64 + MSIZE] = wh
    WhT = np.ascontiguousarray(WhT)
    b_video = np.ascontiguousarray(
        np.asarray(inputs["b_video"], np.float32).reshape(-1, 128).T
    )
    b_audio = np.ascontiguousarray(
        np.asarray(inputs["b_audio"], np.float32).reshape(-1, 128).T
    )

    a2 = audio.reshape(BT, ASIZE).astype(np.float16)
    v2 = video.reshape(BT, MSIZE, VSIZE).astype(np.float16)
    in_maps = []
    for c in range(NCORES):
        sl = slice(c * PER, (c + 1) * PER)
        vT = np.ascontiguousarray(v2[sl].reshape(R, VSIZE).T)
        audioT = np.ascontiguousarray(a2[sl].T)
        in_maps.append(
            {
                "vT": vT,
                "audioT": audioT,
                "WvideoT": WvideoT,
                "WaudioT": WaudioT,
                "WgT": WgT,
                "WvT": WvT,
                "WhT": WhT,
                "b_video": b_video,
                "b_audio": b_audio,
            }
        )
    return in_maps


def _run(inputs, trace=False, **spmd_kwargs):
    from concourse.bass_utils import run_bass_kernel_spmd

    if "nc" not in _cached:
        _cached["nc"] = _build_nc()
    nc = _cached["nc"]
    in_maps = _prep_in_maps(inputs)
    res = run_bass_kernel_spmd(
        nc, in_maps, core_ids=list(range(NCORES)), trace=trace, **spmd_kwargs
    )
    def _part(r):
        denom = r["ezrow"].astype(np.float32).reshape(PER, MSIZE).sum(axis=1)
        return (r["cT"].astype(np.float32) / denom[None, :]).T

    parts = [_part(r) for r in res.results]
    out = np.concatenate(parts, axis=0).reshape(B, T, VSIZE)
    return np.ascontiguousarray(out.astype(np.float32)), res


def kernel(**inputs):
    out, _ = _run(inputs, trace=False)
    return out
